# revision 1
# baseline (speedup 1.0000x reference)
"""Trainium2 Bass kernel for pre-LN multi-head self-attention.

Problem shapes (hardcoded): q (4, 2048, 1024) f32, attn_mask (2048, 2048) bool,
Wq/Wk/Wv (1024, 1024) f32, Wo (1024, 1024) f32, gamma/beta (1024,) f32.
N_HEAD=16, D_HEAD=64, pre-layernorm, softmax over the key axis.

Sharding: 8 cores = 4 batches x 2 head-groups (8 heads each). Each core
computes LN(q_b), its head-slice of the QKV projections, attention for its
8 heads, and a partial O-projection. The O-projection "all-reduce" over the
2 head-groups of a batch plus the qn residual add happens at host gather time.

On-device pipeline per core (all matmuls bf16 with fp32 PSUM accumulation):
  LN(f32, bn_stats)  ->  qn  -> PE-transpose -> qnT (bf16)
  qnT @ W{q,k}  -> hqT/hkT [head_dim, seq]   (per-head slices at partition
                                              offsets 0/64 -> PE row-tiling)
  qnT @ Wv      -> hv [seq, head_dim] augmented with a ones column so the
                   AV matmul also produces the softmax denominators
  S^T[j,i] = hkT_h^T' . hqT_h  (K=64 matmuls, two heads packed in the array)
  P = exp(SCALE*S) (ACT, fused scale) * mask01 (DVE, bf16 2x)
  vecT_aug = hv_aug^T' . P  (accumulated over j chunks; row 64 = denominators)
  vecT *= 1/denom  (reciprocal_approx + ones-column broadcast matmul)
  part = vecT^T' . Wo  -> DMA out (f32);  qn is DMA'd out separately (f32).
"""

import numpy as np
import ml_dtypes
from contextlib import ExitStack

import concourse.bass as bass
import concourse.tile as tile
from concourse import bacc, mybir
from concourse.bass_utils import run_bass_kernel_spmd

F32 = mybir.dt.float32
BF16 = mybir.dt.bfloat16
Alu = mybir.AluOpType
Act = mybir.ActivationFunctionType

BSZ, SEQ, DM = 4, 2048, 1024
NH, DH = 16, 64
HPC = 8              # heads per core
HD = HPC * DH        # 512 = per-core slice of the head dim
NCORES = 8
SCALE = 1.0 / (DH ** 0.5)
LN_EPS = 1e-5

NT = SEQ // 128      # 16 seq tiles of 128
NDC = DM // 128      # 8 d_model chunks of 128
NHC = HD // 128      # 4 per-core head-dim chunks of 128
HVW = DH + 1         # 65: hv columns per head incl. the ones column


def _mha_tile(ctx, tc, dq, dmask, dwq, dwk, dwv, dwo, dident, dgamma, dbeta,
              dqn, dpart, parts="full"):
    nc = tc.nc
    do_qkv = parts in ("qkv", "attn", "full")
    do_attn = parts in ("attn", "full")
    do_o = parts == "full"

    persist = ctx.enter_context(tc.tile_pool(name="persist", bufs=1))
    ident_sb = persist.tile([128, 128], BF16)
    nc.sync.dma_start(out=ident_sb, in_=dident)
    eps_sb = persist.tile([128, 1], F32)
    nc.vector.memset(eps_sb, LN_EPS)

    hqT = persist.tile([128, NHC * SEQ], BF16)      # [hd-chunk part, seq]
    hkT = persist.tile([128, NHC * SEQ], BF16)
    hv = persist.tile([128, NT * HPC * HVW], BF16)  # [j part, per-jc 8*65]
    vecT = persist.tile([128, NHC * SEQ], BF16)     # [hd-chunk part, i]
    wo_sb = persist.tile([128, NHC * DM], BF16)

    gamma_sb = beta_sb = None
    if dgamma is not None:
        gamma_sb = persist.tile([128, DM], F32)
        beta_sb = persist.tile([128, DM], F32)
        nc.sync.dma_start(out=gamma_sb, in_=bass.AP(
            tensor=dgamma.tensor, offset=dgamma.offset,
            ap=[[0, 128]] + list(dgamma.ap)))
        nc.sync.dma_start(out=beta_sb, in_=bass.AP(
            tensor=dbeta.tensor, offset=dbeta.offset,
            ap=[[0, 128]] + list(dbeta.ap)))

    # ---------------- Phase A: LN + transpose + QKV projections -------------
    with tc.tile_pool(name="phA", bufs=1) as pA, \
         tc.tile_pool(name="qtiles", bufs=3) as qpool, \
         tc.tile_pool(name="qnbf", bufs=2) as qnbfpool, \
         tc.tile_pool(name="stats", bufs=6) as spool, \
         tc.tile_pool(name="psT", bufs=2, space="PSUM") as psT, \
         tc.tile_pool(name="psQK", bufs=3, space="PSUM") as psQK:

        qnT = pA.tile([128, NDC * SEQ], BF16)       # [dm-chunk part, seq]
        wq_sb = pA.tile([128, NDC * HD], BF16)
        wk_sb = pA.tile([128, NDC * HD], BF16)
        wv_sb = pA.tile([128, NDC * HD], BF16)

        for tb in range(NT // 4):
            # one merged DMA loads 4 seq tiles; one merged DMA stores 4 qn tiles
            rows4 = slice(tb * 512, (tb + 1) * 512)
            qt = qpool.tile([128, 4, DM], F32, tag="qt")
            nc.sync.dma_start(
                out=qt, in_=dq[rows4, :].rearrange("(a p) m -> p a m", p=128))
            if tb == 0:
                # weights are first needed by QKV(tb=0) ~40us in; issuing them
                # after the first q block keeps LN off the critical path
                for w_sb, dw in ((wq_sb, dwq), (wk_sb, dwk), (wv_sb, dwv)):
                    nc.sync.dma_start(
                        out=w_sb.rearrange("p (dc hd) -> p dc hd", hd=HD),
                        in_=dw.rearrange("(dc p) hd -> p dc hd", p=128))
            qnf = qt        # LN is computed in place: q values are dead after stats
            for k in range(4):
                st = spool.tile([128, 2, 6], F32, tag="st")
                nc.vector.bn_stats(out=st[:, 0, :], in_=qt[:, k, 0:512])
                nc.vector.bn_stats(out=st[:, 1, :], in_=qt[:, k, 512:1024])
                mv = spool.tile([128, 2], F32, tag="mv")
                nc.vector.bn_aggr(out=mv, in_=st)
                # rstd = exp(-0.5*ln(var+eps)) keeps ACT in the exp/ln table set
                lnv = spool.tile([128, 1], F32, tag="lnv")
                nc.scalar.activation(out=lnv, in_=mv[:, 1:2], func=Act.Ln,
                                     bias=eps_sb, scale=1.0)
                rstd = spool.tile([128, 1], F32, tag="rstd")
                nc.scalar.activation(out=rstd, in_=lnv, func=Act.Exp, scale=-0.5)
                negmr = spool.tile([128, 1], F32, tag="negmr")
                nc.vector.tensor_tensor(out=negmr, in0=mv[:, 0:1], in1=rstd,
                                        op=Alu.mult)
                nc.vector.tensor_scalar_mul(negmr, negmr, -1.0)
                nc.vector.tensor_scalar(out=qnf[:, k, :], in0=qt[:, k, :],
                                        scalar1=rstd, scalar2=negmr,
                                        op0=Alu.mult, op1=Alu.add)
                if gamma_sb is not None:
                    nc.vector.tensor_tensor(out=qnf[:, k, :], in0=qnf[:, k, :],
                                            in1=gamma_sb, op=Alu.mult)
                    nc.vector.tensor_tensor(out=qnf[:, k, :], in0=qnf[:, k, :],
                                            in1=beta_sb, op=Alu.add)
            nc.gpsimd.dma_start(
                out=dqn[rows4, :].rearrange("(a p) m -> p a m", p=128), in_=qnf)
            qnbf = qnbfpool.tile([128, 4, DM], BF16, tag="qnbf")
            nc.vector.tensor_copy(out=qnbf, in_=qnf)
            for dc in range(NDC):
                pst = psT.tile([128, 512], BF16, tag="pst")
                for k in range(4):
                    nc.tensor.transpose(pst[:, k * 128:(k + 1) * 128],
                                        qnbf[:, k, dc * 128:(dc + 1) * 128],
                                        ident_sb)
                nc.scalar.copy(
                    out=qnT[:, dc * SEQ + tb * 512: dc * SEQ + (tb + 1) * 512],
                    in_=pst)

            # QKV for this seq block (sc == tb): overlaps the next block's LN
            if not do_qkv:
                continue
            sc = tb
            for w_sb, dstT in ((wq_sb, hqT), (wk_sb, hkT)):
                for hc in range(NHC):
                    ps = psQK.tile([128, 512], F32, tag="psqk")
                    for dc in range(NDC):
                        nc.tensor.matmul(
                            ps,
                            lhsT=w_sb[:, dc * HD + hc * 128: dc * HD + (hc + 1) * 128],
                            rhs=qnT[:, dc * SEQ + sc * 512: dc * SEQ + (sc + 1) * 512],
                            start=(dc == 0), stop=(dc == NDC - 1))
                    nc.vector.tensor_copy(
                        out=dstT[:, hc * SEQ + sc * 512: hc * SEQ + (sc + 1) * 512],
                        in_=ps)
            for jc in range(4 * tb, 4 * tb + 4):
                ps = psQK.tile([128, HD], F32, tag="psv")
                for dc in range(NDC):
                    nc.tensor.matmul(
                        ps,
                        lhsT=qnT[:, dc * SEQ + jc * 128: dc * SEQ + jc * 128 + 128],
                        rhs=wv_sb[:, dc * HD:(dc + 1) * HD],
                        start=(dc == 0), stop=(dc == NDC - 1))
                blk = hv[:, jc * HPC * HVW:(jc + 1) * HPC * HVW]
                blk3 = blk.rearrange("p (h x) -> p h x", x=HVW)
                nc.vector.tensor_copy(out=blk3[:, :, 0:DH],
                                      in_=ps.rearrange("p (h x) -> p h x", x=DH))
        hv4 = hv.rearrange("p (j h x) -> p j h x", h=HPC, x=HVW)
        nc.vector.memset(hv4[:, :, :, DH:HVW], 1.0)

    # ---------------- Phase B: attention (two heads packed per pass) --------
    drecip = nc.dram_tensor(f"recip_scratch{nc.next_id()}", [HPC, SEQ], F32).ap()
    with tc.tile_pool(name="mk", bufs=1) as mkpool, \
         tc.tile_pool(name="pp", bufs=3) as ppool, \
         tc.tile_pool(name="stg", bufs=1) as stpool, \
         tc.tile_pool(name="den", bufs=1) as denpool, \
         tc.tile_pool(name="sps", bufs=1, space="PSUM") as spsum, \
         tc.tile_pool(name="vps", bufs=1, space="PSUM") as vpsum:
        # whole mask resident: [j, i] in jc column blocks, 8 chunked DMAs
        mask_all = mkpool.tile([128, NT * SEQ], BF16)
        for c in range(8):
            nc.scalar.dma_start(
                out=mask_all[:, c * 2 * SEQ:(c + 1) * 2 * SEQ]
                    .rearrange("p (a i) -> p a i", i=SEQ),
                in_=dmask[c * 256:(c + 1) * 256, :]
                    .rearrange("(a p) i -> p a i", p=128))
        nc.scalar.dma_start(
            out=wo_sb.rearrange("p (hc m) -> p hc m", m=DM),
            in_=dwo.rearrange("(hc p) m -> p hc m", p=128))
        if not do_attn:
            nc.vector.memset(vecT, 0.0)
        for hp in range(HPC // 2 if do_attn else 0):
            ha, hb = 2 * hp, 2 * hp + 1
            den_hp = denpool.tile([2, SEQ], F32, tag="den")
            for ih in range(2):
                vA = vpsum.tile([65, 1024], F32, tag="vA")
                vB = vpsum.tile([65, 1024], F32, tag="vB")
                for jc in range(NT):
                    mk = mask_all[:, jc * SEQ + ih * 1024: jc * SEQ + (ih + 1) * 1024]
                    sA = spsum.tile([128, 1024], F32, tag="sA")
                    sB = spsum.tile([128, 1024], F32, tag="sB")
                    kslice = slice(hp * SEQ + jc * 128, hp * SEQ + (jc + 1) * 128)
                    for n in range(2):
                        qsl = slice(hp * SEQ + ih * 1024 + n * 512,
                                    hp * SEQ + ih * 1024 + (n + 1) * 512)
                        osl = slice(n * 512, (n + 1) * 512)
                        nc.tensor.matmul(sA[:, osl], lhsT=hkT[0:64, kslice],
                                         rhs=hqT[0:64, qsl], start=True, stop=True)
                        nc.tensor.matmul(sB[:, osl], lhsT=hkT[64:128, kslice],
                                         rhs=hqT[64:128, qsl], start=True, stop=True)
                    pa = ppool.tile([128, 1024], BF16, tag="pa")
                    pb = ppool.tile([128, 1024], BF16, tag="pb")
                    nc.scalar.activation(out=pa, in_=sA, func=Act.Exp, scale=SCALE)
                    nc.scalar.activation(out=pb, in_=sB, func=Act.Exp, scale=SCALE)
                    nc.vector.tensor_mul(pa, pa, mk)
                    nc.vector.tensor_mul(pb, pb, mk)
                    va_l = hv[:, jc * HPC * HVW + ha * HVW: jc * HPC * HVW + ha * HVW + HVW]
                    vb_l = hv[:, jc * HPC * HVW + hb * HVW: jc * HPC * HVW + hb * HVW + HVW]
                    for n in range(2):
                        osl = slice(n * 512, (n + 1) * 512)
                        nc.tensor.matmul(vA[:, osl], lhsT=va_l, rhs=pa[:, osl],
                                         start=(jc == 0), stop=(jc == NT - 1))
                        nc.tensor.matmul(vB[:, osl], lhsT=vb_l, rhs=pb[:, osl],
                                         start=(jc == 0), stop=(jc == NT - 1))
                isl = slice(hp * SEQ + ih * 1024, hp * SEQ + (ih + 1) * 1024)
                dsl = slice(ih * 1024, (ih + 1) * 1024)
                # denominators (psum row 64) are staged to SBUF (DMA cannot
                # read PSUM) then DMA'd to den_hp partitions 0/1; head-b vec
                # rows must move from psum partitions 0:64 to sbuf 64:128 --
                # both are cross-partition moves only a DMA can do.
                dsa = stpool.tile([65, 1024], F32, tag="dsa")
                dsb = stpool.tile([65, 1024], F32, tag="dsb")
                nc.vector.tensor_copy(out=dsa[64:65, :], in_=vA[64:65, :])
                nc.vector.tensor_copy(out=dsb[64:65, :], in_=vB[64:65, :])
                nc.sync.dma_start(out=den_hp[0:1, dsl], in_=dsa[64:65, :])
                nc.sync.dma_start(out=den_hp[1:2, dsl], in_=dsb[64:65, :])
                nc.vector.tensor_copy(out=vecT[0:64, isl], in_=vA[0:64, :])
                stage = stpool.tile([64, 1024], BF16, tag="stg")
                nc.vector.tensor_copy(out=stage, in_=vB[0:64, :])
                nc.gpsimd.dma_start(out=vecT[64:128, isl], in_=stage)
            # normalize this head pair while the next pair's attention runs
            recip_hp = denpool.tile([2, SEQ], F32, tag="recip")
            nc.vector.reciprocal_approx_fast(out=recip_hp, in_=den_hp)
            nc.sync.dma_start(out=drecip[ha:hb + 1, :], in_=recip_hp)
            bc_hp = denpool.tile([128, SEQ], F32, tag="bc")
            for h, lo in ((ha, 0), (hb, 64)):
                row = drecip[h:h + 1, :]
                nc.sync.dma_start(
                    out=bc_hp[lo:lo + 64, :],
                    in_=bass.AP(tensor=row.tensor, offset=row.offset,
                                ap=[[0, 64]] + list(row.ap[1:])))
            hsl = slice(hp * SEQ, (hp + 1) * SEQ)
            nc.vector.tensor_tensor(out=vecT[:, hsl], in0=vecT[:, hsl],
                                    in1=bc_hp, op=Alu.mult)

    # ---------------- Phase C: O-projection ---------------------------------
    with tc.tile_pool(name="po", bufs=4, space="PSUM") as opool, \
         tc.tile_pool(name="outs", bufs=2) as outpool:
        for ob in range(NT // 4):
            outt = outpool.tile([128, 4, DM], F32, tag="outt")
            if not do_o:
                nc.vector.memset(outt, 0.0)
                nc.sync.dma_start(
                    out=dpart[ob * 512:(ob + 1) * 512, :]
                        .rearrange("(a p) m -> p a m", p=128),
                    in_=outt)
                continue
            for k in range(4):
                it = ob * 4 + k
                for mc in range(2):
                    po = opool.tile([128, 512], F32, tag="po")
                    for hc in range(NHC):
                        nc.tensor.matmul(
                            po,
                            lhsT=vecT[:, hc * SEQ + it * 128: hc * SEQ + (it + 1) * 128],
                            rhs=wo_sb[:, hc * DM + mc * 512: hc * DM + (mc + 1) * 512],
                            start=(hc == 0), stop=(hc == NHC - 1))
                    nc.vector.tensor_copy(out=outt[:, k, mc * 512:(mc + 1) * 512],
                                          in_=po)
            nc.sync.dma_start(
                out=dpart[ob * 512:(ob + 1) * 512, :]
                    .rearrange("(a p) m -> p a m", p=128),
                in_=outt)


_NC_CACHE = {}


def _build(gamma_trivial, repeat=1, parts="full"):
    key = (bool(gamma_trivial), repeat, parts)
    if key in _NC_CACHE:
        return _NC_CACHE[key]
    nc = bacc.Bacc("TRN2", target_bir_lowering=False, debug=False,
                   num_devices=NCORES)
    dq = nc.dram_tensor("q", [SEQ, DM], F32, kind="ExternalInput").ap()
    dmask = nc.dram_tensor("maskt", [SEQ, SEQ], BF16, kind="ExternalInput").ap()
    dwq = nc.dram_tensor("wq", [DM, HD], BF16, kind="ExternalInput").ap()
    dwk = nc.dram_tensor("wk", [DM, HD], BF16, kind="ExternalInput").ap()
    dwv = nc.dram_tensor("wv", [DM, HD], BF16, kind="ExternalInput").ap()
    dwo = nc.dram_tensor("wo", [HD, DM], BF16, kind="ExternalInput").ap()
    dident = nc.dram_tensor("ident", [128, 128], BF16, kind="ExternalInput").ap()
    dgamma = dbeta = None
    if not gamma_trivial:
        dgamma = nc.dram_tensor("gamma", [DM], F32, kind="ExternalInput").ap()
        dbeta = nc.dram_tensor("beta", [DM], F32, kind="ExternalInput").ap()
    dqn = nc.dram_tensor("qn_out", [SEQ, DM], F32, kind="ExternalOutput").ap()
    dpart = nc.dram_tensor("part_out", [SEQ, DM], F32, kind="ExternalOutput").ap()
    with tile.TileContext(nc) as tc:
        for _rep in range(repeat):
            with ExitStack() as ctx:
                _mha_tile(ctx, tc, dq, dmask, dwq, dwk, dwv, dwo, dident,
                          dgamma, dbeta, dqn, dpart, parts=parts)
    nc.compile()
    _NC_CACHE[key] = nc
    return nc


def _run(nc, in_maps, **kwargs):
    return run_bass_kernel_spmd(nc, in_maps, list(range(NCORES)), **kwargs)


def make_in_maps(q, attn_mask, Wq, Wk, Wv, Wo, gamma, beta, gamma_trivial):
    bf = ml_dtypes.bfloat16
    q = np.ascontiguousarray(np.asarray(q, dtype=np.float32))
    maskt = np.ascontiguousarray(
        (~np.asarray(attn_mask, dtype=bool)).T.astype(bf))
    Wq = np.asarray(Wq, dtype=np.float32)
    Wk = np.asarray(Wk, dtype=np.float32)
    Wv = np.asarray(Wv, dtype=np.float32)
    Wo = np.asarray(Wo, dtype=np.float32)
    ident = np.eye(128, dtype=bf)
    in_maps = []
    for c in range(NCORES):
        b, g = c // 2, c % 2
        cols = slice(g * HD, (g + 1) * HD)
        m = {
            "q": q[b],
            "maskt": maskt,
            "wq": np.ascontiguousarray(Wq[:, cols].astype(bf)),
            "wk": np.ascontiguousarray(Wk[:, cols].astype(bf)),
            "wv": np.ascontiguousarray(Wv[:, cols].astype(bf)),
            "wo": np.ascontiguousarray(Wo[cols, :].astype(bf)),
            "ident": ident,
        }
        if not gamma_trivial:
            m["gamma"] = np.asarray(gamma, dtype=np.float32)
            m["beta"] = np.asarray(beta, dtype=np.float32)
        in_maps.append(m)
    return in_maps


def kernel(q, attn_mask, Wq, Wk, Wv, Wo, gamma, beta):
    gamma_np = np.asarray(gamma, dtype=np.float32)
    beta_np = np.asarray(beta, dtype=np.float32)
    gamma_trivial = bool(np.all(gamma_np == 1.0) and np.all(beta_np == 0.0))
    nc = _build(gamma_trivial)
    in_maps = make_in_maps(q, attn_mask, Wq, Wk, Wv, Wo, gamma_np, beta_np,
                           gamma_trivial)
    res = _run(nc, in_maps).results
    out = np.empty((BSZ, SEQ, DM), dtype=np.float32)
    for b in range(BSZ):
        out[b] = res[2 * b]["qn_out"]
        out[b] += res[2 * b]["part_out"]
        out[b] += res[2 * b + 1]["part_out"]
    return out


if __name__ == "__main__":
    rng = np.random.default_rng(0)
    ins = {
        "q": rng.standard_normal((BSZ, SEQ, DM), dtype=np.float32),
        "attn_mask": rng.integers(0, 2, (SEQ, SEQ)).astype(bool),
        "Wq": rng.standard_normal((DM, NH * DH), dtype=np.float32) * 0.03,
        "Wk": rng.standard_normal((DM, NH * DH), dtype=np.float32) * 0.03,
        "Wv": rng.standard_normal((DM, NH * DH), dtype=np.float32) * 0.03,
        "Wo": rng.standard_normal((NH * DH, DM), dtype=np.float32) * 0.03,
        "gamma": np.ones(DM, np.float32),
        "beta": np.zeros(DM, np.float32),
    }
    out = kernel(**ins)
    print("kernel ran, out shape", out.shape, out.dtype)



# revision 2
# speedup vs baseline: 1.1033x; 1.1033x over previous
"""Trainium2 Bass kernel for pre-LN multi-head self-attention.

Problem shapes (hardcoded): q (4, 2048, 1024) f32, attn_mask (2048, 2048) bool,
Wq/Wk/Wv (1024, 1024) f32, Wo (1024, 1024) f32, gamma/beta (1024,) f32.
N_HEAD=16, D_HEAD=64, pre-layernorm, softmax over the key axis.

Sharding: 8 cores = 4 batches x 2 head-groups (8 heads each). Each core
computes LN(q_b), its head-slice of the QKV projections, attention for its
8 heads, and a partial O-projection. The O-projection "all-reduce" over the
2 head-groups of a batch plus the qn residual add happens at host gather time.

v2 restructure (from trace analysis of the 591us baseline):
  - LN rstd via ACT Sqrt + DVE reciprocal_approx_fast: one ACT table set for
    all of phase A (the Ln/Exp pair alternated sets -> 29 table reloads).
  - qn is bf16 end-to-end (TS writes bf16, qn_out is bf16, host upcasts);
    kills the 61us f32->bf16 CAST pass.
  - QKV psum->sbuf copies moved to ScalarE (idle in phase A; DVE was 111us).
  - Phase B software-pipelined: per jc the PE issue order is QK(jc) then
    AV(jc-1), so the serial QK->exp->mask->AV chain (3.4us/jc measured)
    becomes ACT-bound ping-pong (~2.3-2.5us/jc): exp(head a, jc) frees sA
    while exp(head b, jc) runs, QK(jc+1) slots under it.
  - pa/pb in one [128,2048] tile; one mask TENSOR_TENSOR with a stride-0
    broadcast AP over the head dim instead of two.
  - mask resident as [128, 16, 2048] loaded in 16 chunk DMAs at phase B
    start (keeps phase A under the SBUF cap).
"""

import numpy as np
import ml_dtypes
from contextlib import ExitStack

import concourse.bass as bass
import concourse.tile as tile
from concourse import bacc, mybir
from concourse.bass_utils import run_bass_kernel_spmd

F32 = mybir.dt.float32
BF16 = mybir.dt.bfloat16
Alu = mybir.AluOpType
Act = mybir.ActivationFunctionType

BSZ, SEQ, DM = 4, 2048, 1024
NH, DH = 16, 64
HPC = 8              # heads per core
HD = HPC * DH        # 512 = per-core slice of the head dim
NCORES = 8
SCALE = 1.0 / (DH ** 0.5)
LN_EPS = 1e-5

NT = SEQ // 128      # 16 seq tiles of 128
NDC = DM // 128      # 8 d_model chunks of 128
NHC = HD // 128      # 4 per-core head-dim chunks of 128
HVW = DH + 1         # 65: hv columns per head incl. the ones column


def _mha_tile(ctx, tc, dq, dmask, dwq, dwk, dwv, dwo, dident, dgamma, dbeta,
              dqn, dpart, parts="full"):
    nc = tc.nc
    do_qkv = parts in ("qkv", "attn", "full")
    do_attn = parts in ("attn", "full")
    do_o = parts == "full"

    persist = ctx.enter_context(tc.tile_pool(name="persist", bufs=1))
    ident_sb = persist.tile([128, 128], BF16)
    nc.sync.dma_start(out=ident_sb, in_=dident)
    eps_sb = persist.tile([128, 1], F32)
    nc.vector.memset(eps_sb, LN_EPS)

    hqT = persist.tile([128, NHC * SEQ], BF16)      # [hd-chunk part, seq]
    hkT = persist.tile([128, NHC * SEQ], BF16)
    hv = persist.tile([128, NT * HPC * HVW], BF16)  # [j part, per-jc 8*65]
    vecT = persist.tile([128, NHC * SEQ], BF16)     # [hd-chunk part, i]
    wo_sb = persist.tile([128, NHC * DM], BF16)

    gamma_sb = beta_sb = None
    if dgamma is not None:
        gamma_sb = persist.tile([128, DM], F32)
        beta_sb = persist.tile([128, DM], F32)
        nc.sync.dma_start(out=gamma_sb, in_=bass.AP(
            tensor=dgamma.tensor, offset=dgamma.offset,
            ap=[[0, 128]] + list(dgamma.ap)))
        nc.sync.dma_start(out=beta_sb, in_=bass.AP(
            tensor=dbeta.tensor, offset=dbeta.offset,
            ap=[[0, 128]] + list(dbeta.ap)))

    # ---------------- Phase A: LN + transpose + QKV projections -------------
    with tc.tile_pool(name="phA", bufs=1) as pA, \
         tc.tile_pool(name="qtiles", bufs=2) as qpool, \
         tc.tile_pool(name="qnbf", bufs=2) as qnbfpool, \
         tc.tile_pool(name="stats", bufs=6) as spool, \
         tc.tile_pool(name="psT", bufs=2, space="PSUM") as psT, \
         tc.tile_pool(name="psQK", bufs=3, space="PSUM") as psQK:

        qnT = pA.tile([128, NDC * SEQ], BF16)       # [dm-chunk part, seq]
        wq_sb = pA.tile([128, NDC * HD], BF16)
        wk_sb = pA.tile([128, NDC * HD], BF16)
        wv_sb = pA.tile([128, NDC * HD], BF16)

        for tb in range(NT // 4):
            # one merged DMA loads 4 seq tiles
            rows4 = slice(tb * 512, (tb + 1) * 512)
            qt = qpool.tile([128, 4, DM], F32, tag="qt")
            nc.sync.dma_start(
                out=qt, in_=dq[rows4, :].rearrange("(a p) m -> p a m", p=128))
            if tb == 0:
                # weights are first needed by QKV(tb=0) ~30us in
                for w_sb, dw in ((wq_sb, dwq), (wk_sb, dwk), (wv_sb, dwv)):
                    nc.sync.dma_start(
                        out=w_sb.rearrange("p (dc hd) -> p dc hd", hd=HD),
                        in_=dw.rearrange("(dc p) hd -> p dc hd", p=128))
            qnbf = qnbfpool.tile([128, 4, DM], BF16, tag="qnbf")
            for k in range(4):
                st = spool.tile([128, 2, 6], F32, tag="st")
                nc.vector.bn_stats(out=st[:, 0, :], in_=qt[:, k, 0:512])
                nc.vector.bn_stats(out=st[:, 1, :], in_=qt[:, k, 512:1024])
                mv = spool.tile([128, 2], F32, tag="mv")
                nc.vector.bn_aggr(out=mv, in_=st)
                # rstd = 1/sqrt(var+eps): Sqrt on ACT (single table set for
                # all of phase A), reciprocal on DVE (~18-bit, plenty).
                std = spool.tile([128, 1], F32, tag="std")
                nc.scalar.activation(out=std, in_=mv[:, 1:2], func=Act.Sqrt,
                                     bias=eps_sb, scale=1.0)
                rstd = spool.tile([128, 1], F32, tag="rstd")
                nc.vector.reciprocal_approx_fast(out=rstd, in_=std)
                negmr = spool.tile([128, 1], F32, tag="negmr")
                nc.vector.tensor_tensor(out=negmr, in0=mv[:, 0:1], in1=rstd,
                                        op=Alu.mult)
                nc.vector.tensor_scalar_mul(negmr, negmr, -1.0)
                if gamma_sb is None:
                    nc.vector.tensor_scalar(out=qnbf[:, k, :], in0=qt[:, k, :],
                                            scalar1=rstd, scalar2=negmr,
                                            op0=Alu.mult, op1=Alu.add)
                else:
                    qnf = qt
                    nc.vector.tensor_scalar(out=qnf[:, k, :], in0=qt[:, k, :],
                                            scalar1=rstd, scalar2=negmr,
                                            op0=Alu.mult, op1=Alu.add)
                    nc.vector.tensor_tensor(out=qnf[:, k, :], in0=qnf[:, k, :],
                                            in1=gamma_sb, op=Alu.mult)
                    nc.vector.tensor_tensor(out=qnbf[:, k, :], in0=qnf[:, k, :],
                                            in1=beta_sb, op=Alu.add)
            nc.gpsimd.dma_start(
                out=dqn[rows4, :].rearrange("(a p) m -> p a m", p=128), in_=qnbf)
            for dc in range(NDC):
                pst = psT.tile([128, 512], BF16, tag="pst")
                for k in range(4):
                    nc.tensor.transpose(pst[:, k * 128:(k + 1) * 128],
                                        qnbf[:, k, dc * 128:(dc + 1) * 128],
                                        ident_sb)
                nc.scalar.copy(
                    out=qnT[:, dc * SEQ + tb * 512: dc * SEQ + (tb + 1) * 512],
                    in_=pst)

            # QKV for this seq block (sc == tb): overlaps the next block's LN
            if not do_qkv:
                continue
            sc = tb
            for w_sb, dstT in ((wq_sb, hqT), (wk_sb, hkT)):
                for hc in range(NHC):
                    ps = psQK.tile([128, 512], F32, tag="psqk")
                    for dc in range(NDC):
                        nc.tensor.matmul(
                            ps,
                            lhsT=w_sb[:, dc * HD + hc * 128: dc * HD + (hc + 1) * 128],
                            rhs=qnT[:, dc * SEQ + sc * 512: dc * SEQ + (sc + 1) * 512],
                            start=(dc == 0), stop=(dc == NDC - 1))
                    nc.scalar.copy(
                        out=dstT[:, hc * SEQ + sc * 512: hc * SEQ + (sc + 1) * 512],
                        in_=ps)
            for jc in range(4 * tb, 4 * tb + 4):
                ps = psQK.tile([128, HD], F32, tag="psv")
                for dc in range(NDC):
                    nc.tensor.matmul(
                        ps,
                        lhsT=qnT[:, dc * SEQ + jc * 128: dc * SEQ + jc * 128 + 128],
                        rhs=wv_sb[:, dc * HD:(dc + 1) * HD],
                        start=(dc == 0), stop=(dc == NDC - 1))
                blk = hv[:, jc * HPC * HVW:(jc + 1) * HPC * HVW]
                blk3 = blk.rearrange("p (h x) -> p h x", x=HVW)
                nc.scalar.copy(out=blk3[:, :, 0:DH],
                               in_=ps.rearrange("p (h x) -> p h x", x=DH))
        hv4 = hv.rearrange("p (j h x) -> p j h x", h=HPC, x=HVW)
        nc.vector.memset(hv4[:, :, :, DH:HVW], 1.0)

    # ---------------- Phase B: attention, ACT-bound ping-pong ---------------
    drecip = nc.dram_tensor(f"recip_scratch{nc.next_id()}", [HPC, SEQ], F32).ap()
    with tc.tile_pool(name="mk", bufs=1) as mkpool, \
         tc.tile_pool(name="pp", bufs=3) as ppool, \
         tc.tile_pool(name="stg", bufs=2) as stpool, \
         tc.tile_pool(name="den", bufs=1) as denpool, \
         tc.tile_pool(name="sps", bufs=1, space="PSUM") as spsum, \
         tc.tile_pool(name="vps", bufs=1, space="PSUM") as vpsum:
        # whole mask resident [j-part, jc, i]; 16 chunk DMAs stream in jc order
        mask_all = mkpool.tile([128, NT, SEQ], BF16)
        for c in range(NT):
            nc.sync.dma_start(out=mask_all[:, c, :],
                              in_=dmask[c * 128:(c + 1) * 128, :])
        nc.sync.dma_start(
            out=wo_sb.rearrange("p (hc m) -> p hc m", m=DM),
            in_=dwo.rearrange("(hc p) m -> p hc m", p=128))
        if not do_attn:
            nc.vector.memset(vecT, 0.0)

        def issue_av(pab, jc, vA, vB, ha, hb):
            va_l = hv[:, jc * HPC * HVW + ha * HVW: jc * HPC * HVW + ha * HVW + HVW]
            vb_l = hv[:, jc * HPC * HVW + hb * HVW: jc * HPC * HVW + hb * HVW + HVW]
            for n in range(2):
                osl = slice(n * 512, (n + 1) * 512)
                nc.tensor.matmul(vA[:, osl], lhsT=va_l, rhs=pab[:, n * 512:(n + 1) * 512],
                                 start=(jc == 0), stop=(jc == NT - 1))
                nc.tensor.matmul(vB[:, osl], lhsT=vb_l,
                                 rhs=pab[:, 1024 + n * 512: 1024 + (n + 1) * 512],
                                 start=(jc == 0), stop=(jc == NT - 1))

        for hp in range(HPC // 2 if do_attn else 0):
            ha, hb = 2 * hp, 2 * hp + 1
            den_hp = denpool.tile([2, SEQ], F32, tag="den")
            for ih in range(2):
                vA = vpsum.tile([65, 1024], F32, tag="vA")
                vB = vpsum.tile([65, 1024], F32, tag="vB")
                prev = None
                for jc in range(NT):
                    sA = spsum.tile([128, 1024], F32, tag="sA")
                    sB = spsum.tile([128, 1024], F32, tag="sB")
                    kslice = slice(hp * SEQ + jc * 128, hp * SEQ + (jc + 1) * 128)
                    for n in range(2):
                        qsl = slice(hp * SEQ + ih * 1024 + n * 512,
                                    hp * SEQ + ih * 1024 + (n + 1) * 512)
                        osl = slice(n * 512, (n + 1) * 512)
                        nc.tensor.matmul(sA[:, osl], lhsT=hkT[0:64, kslice],
                                         rhs=hqT[0:64, qsl], start=True, stop=True)
                        nc.tensor.matmul(sB[:, osl], lhsT=hkT[64:128, kslice],
                                         rhs=hqT[64:128, qsl], start=True, stop=True)
                    # PE pipeline: AV for jc-1 sits behind QK(jc); it becomes
                    # ready (mask done) while exp(jc) occupies ACT.
                    if prev is not None:
                        issue_av(prev[0], prev[1], vA, vB, ha, hb)
                    pab = ppool.tile([128, 2048], BF16, tag="pab")
                    nc.scalar.activation(out=pab[:, 0:1024], in_=sA,
                                         func=Act.Exp, scale=SCALE)
                    nc.scalar.activation(out=pab[:, 1024:2048], in_=sB,
                                         func=Act.Exp, scale=SCALE)
                    mk = mask_all[:, jc, ih * 1024:(ih + 1) * 1024]
                    mk_bc = bass.AP(tensor=mk.tensor, offset=mk.offset,
                                    ap=[list(mk.ap[0]), [0, 2]] + [list(a) for a in mk.ap[1:]])
                    nc.vector.tensor_tensor(out=pab, in0=pab, in1=mk_bc,
                                            op=Alu.mult)
                    prev = (pab, jc)
                issue_av(prev[0], prev[1], vA, vB, ha, hb)
                isl = slice(hp * SEQ + ih * 1024, hp * SEQ + (ih + 1) * 1024)
                dsl = slice(ih * 1024, (ih + 1) * 1024)
                # denominators (psum row 64) staged to SBUF (DMA cannot read
                # PSUM) then DMA'd to den_hp partitions 0/1; head-b vec rows
                # must move from psum partitions 0:64 to sbuf 64:128 -- both
                # are cross-partition moves only a DMA can do.
                dsa = stpool.tile([65, 1024], F32, tag="dsa")
                dsb = stpool.tile([65, 1024], F32, tag="dsb")
                nc.vector.tensor_copy(out=dsa[64:65, :], in_=vA[64:65, :])
                nc.vector.tensor_copy(out=dsb[64:65, :], in_=vB[64:65, :])
                nc.sync.dma_start(out=den_hp[0:1, dsl], in_=dsa[64:65, :])
                nc.sync.dma_start(out=den_hp[1:2, dsl], in_=dsb[64:65, :])
                nc.vector.tensor_copy(out=vecT[0:64, isl], in_=vA[0:64, :])
                stage = stpool.tile([64, 1024], BF16, tag="stg")
                nc.vector.tensor_copy(out=stage, in_=vB[0:64, :])
                nc.gpsimd.dma_start(out=vecT[64:128, isl], in_=stage)
            # normalize this head pair while the next pair's attention runs
            recip_hp = denpool.tile([2, SEQ], F32, tag="recip")
            nc.vector.reciprocal_approx_fast(out=recip_hp, in_=den_hp)
            nc.sync.dma_start(out=drecip[ha:hb + 1, :], in_=recip_hp)
            bc_hp = denpool.tile([128, SEQ], F32, tag="bc")
            for h, lo in ((ha, 0), (hb, 64)):
                row = drecip[h:h + 1, :]
                nc.sync.dma_start(
                    out=bc_hp[lo:lo + 64, :],
                    in_=bass.AP(tensor=row.tensor, offset=row.offset,
                                ap=[[0, 64]] + list(row.ap[1:])))
            hsl = slice(hp * SEQ, (hp + 1) * SEQ)
            nc.vector.tensor_tensor(out=vecT[:, hsl], in0=vecT[:, hsl],
                                    in1=bc_hp, op=Alu.mult)

    # ---------------- Phase C: O-projection ---------------------------------
    with tc.tile_pool(name="po", bufs=4, space="PSUM") as opool, \
         tc.tile_pool(name="outs", bufs=2) as outpool:
        for ob in range(NT // 4):
            outt = outpool.tile([128, 4, DM], F32, tag="outt")
            if not do_o:
                nc.vector.memset(outt, 0.0)
                nc.sync.dma_start(
                    out=dpart[ob * 512:(ob + 1) * 512, :]
                        .rearrange("(a p) m -> p a m", p=128),
                    in_=outt)
                continue
            for k in range(4):
                it = ob * 4 + k
                for mc in range(2):
                    po = opool.tile([128, 512], F32, tag="po")
                    for hc in range(NHC):
                        nc.tensor.matmul(
                            po,
                            lhsT=vecT[:, hc * SEQ + it * 128: hc * SEQ + (it + 1) * 128],
                            rhs=wo_sb[:, hc * DM + mc * 512: hc * DM + (mc + 1) * 512],
                            start=(hc == 0), stop=(hc == NHC - 1))
                    nc.vector.tensor_copy(out=outt[:, k, mc * 512:(mc + 1) * 512],
                                          in_=po)
            nc.sync.dma_start(
                out=dpart[ob * 512:(ob + 1) * 512, :]
                    .rearrange("(a p) m -> p a m", p=128),
                in_=outt)


_NC_CACHE = {}


def _build(gamma_trivial, repeat=1, parts="full"):
    key = (bool(gamma_trivial), repeat, parts)
    if key in _NC_CACHE:
        return _NC_CACHE[key]
    nc = bacc.Bacc("TRN2", target_bir_lowering=False, debug=False,
                   num_devices=NCORES)
    dq = nc.dram_tensor("q", [SEQ, DM], F32, kind="ExternalInput").ap()
    dmask = nc.dram_tensor("maskt", [SEQ, SEQ], BF16, kind="ExternalInput").ap()
    dwq = nc.dram_tensor("wq", [DM, HD], BF16, kind="ExternalInput").ap()
    dwk = nc.dram_tensor("wk", [DM, HD], BF16, kind="ExternalInput").ap()
    dwv = nc.dram_tensor("wv", [DM, HD], BF16, kind="ExternalInput").ap()
    dwo = nc.dram_tensor("wo", [HD, DM], BF16, kind="ExternalInput").ap()
    dident = nc.dram_tensor("ident", [128, 128], BF16, kind="ExternalInput").ap()
    dgamma = dbeta = None
    if not gamma_trivial:
        dgamma = nc.dram_tensor("gamma", [DM], F32, kind="ExternalInput").ap()
        dbeta = nc.dram_tensor("beta", [DM], F32, kind="ExternalInput").ap()
    dqn = nc.dram_tensor("qn_out", [SEQ, DM], BF16, kind="ExternalOutput").ap()
    dpart = nc.dram_tensor("part_out", [SEQ, DM], F32, kind="ExternalOutput").ap()
    with tile.TileContext(nc) as tc:
        for _rep in range(repeat):
            with ExitStack() as ctx:
                _mha_tile(ctx, tc, dq, dmask, dwq, dwk, dwv, dwo, dident,
                          dgamma, dbeta, dqn, dpart, parts=parts)
    nc.compile()
    _NC_CACHE[key] = nc
    return nc


def _run(nc, in_maps, **kwargs):
    return run_bass_kernel_spmd(nc, in_maps, list(range(NCORES)), **kwargs)


def make_in_maps(q, attn_mask, Wq, Wk, Wv, Wo, gamma, beta, gamma_trivial):
    bf = ml_dtypes.bfloat16
    q = np.ascontiguousarray(np.asarray(q, dtype=np.float32))
    maskt = np.ascontiguousarray(
        (~np.asarray(attn_mask, dtype=bool)).T.astype(bf))
    Wq = np.asarray(Wq, dtype=np.float32)
    Wk = np.asarray(Wk, dtype=np.float32)
    Wv = np.asarray(Wv, dtype=np.float32)
    Wo = np.asarray(Wo, dtype=np.float32)
    ident = np.eye(128, dtype=bf)
    in_maps = []
    for c in range(NCORES):
        b, g = c // 2, c % 2
        cols = slice(g * HD, (g + 1) * HD)
        m = {
            "q": q[b],
            "maskt": maskt,
            "wq": np.ascontiguousarray(Wq[:, cols].astype(bf)),
            "wk": np.ascontiguousarray(Wk[:, cols].astype(bf)),
            "wv": np.ascontiguousarray(Wv[:, cols].astype(bf)),
            "wo": np.ascontiguousarray(Wo[cols, :].astype(bf)),
            "ident": ident,
        }
        if not gamma_trivial:
            m["gamma"] = np.asarray(gamma, dtype=np.float32)
            m["beta"] = np.asarray(beta, dtype=np.float32)
        in_maps.append(m)
    return in_maps


def kernel(q, attn_mask, Wq, Wk, Wv, Wo, gamma, beta):
    gamma_np = np.asarray(gamma, dtype=np.float32)
    beta_np = np.asarray(beta, dtype=np.float32)
    gamma_trivial = bool(np.all(gamma_np == 1.0) and np.all(beta_np == 0.0))
    nc = _build(gamma_trivial)
    in_maps = make_in_maps(q, attn_mask, Wq, Wk, Wv, Wo, gamma_np, beta_np,
                           gamma_trivial)
    res = _run(nc, in_maps).results
    out = np.empty((BSZ, SEQ, DM), dtype=np.float32)
    for b in range(BSZ):
        out[b] = res[2 * b]["qn_out"].astype(np.float32)
        out[b] += res[2 * b]["part_out"]
        out[b] += res[2 * b + 1]["part_out"]
    return out


if __name__ == "__main__":
    rng = np.random.default_rng(0)
    ins = {
        "q": rng.standard_normal((BSZ, SEQ, DM), dtype=np.float32),
        "attn_mask": rng.integers(0, 2, (SEQ, SEQ)).astype(bool),
        "Wq": rng.standard_normal((DM, NH * DH), dtype=np.float32) * 0.03,
        "Wk": rng.standard_normal((DM, NH * DH), dtype=np.float32) * 0.03,
        "Wv": rng.standard_normal((DM, NH * DH), dtype=np.float32) * 0.03,
        "Wo": rng.standard_normal((NH * DH, DM), dtype=np.float32) * 0.03,
        "gamma": np.ones(DM, np.float32),
        "beta": np.zeros(DM, np.float32),
    }
    out = kernel(**ins)
    print("kernel ran, out shape", out.shape, out.dtype)


# revision 7
# speedup vs baseline: 1.1455x; 1.0383x over previous
"""Trainium2 Bass kernel for pre-LN multi-head self-attention.

Problem shapes (hardcoded): q (4, 2048, 1024) f32, attn_mask (2048, 2048) bool,
Wq/Wk/Wv (1024, 1024) f32, Wo (1024, 1024) f32, gamma/beta (1024,) f32.
N_HEAD=16, D_HEAD=64, pre-layernorm, softmax over the key axis.

Sharding: 8 cores = 4 batches x 2 head-groups (8 heads each). Each core
computes LN(q_b), its head-slice of the QKV projections, attention for its
8 heads, and a partial O-projection. The O-projection "all-reduce" over the
2 head-groups of a batch plus the qn residual add happens at host gather time.

v2 restructure (from trace analysis of the 591us baseline):
  - LN rstd via ACT Sqrt + DVE reciprocal_approx_fast: one ACT table set for
    all of phase A (the Ln/Exp pair alternated sets -> 29 table reloads).
  - qn is bf16 end-to-end (TS writes bf16, qn_out is bf16, host upcasts);
    kills the 61us f32->bf16 CAST pass.
  - QKV psum->sbuf copies moved to ScalarE (idle in phase A; DVE was 111us).
  - Phase B software-pipelined: per jc the PE issue order is QK(jc) then
    AV(jc-1), so the serial QK->exp->mask->AV chain (3.4us/jc measured)
    becomes ACT-bound ping-pong (~2.3-2.5us/jc): exp(head a, jc) frees sA
    while exp(head b, jc) runs, QK(jc+1) slots under it.
  - pa/pb in one [128,2048] tile; one mask TENSOR_TENSOR with a stride-0
    broadcast AP over the head dim instead of two.
  - mask resident as [128, 16, 2048] loaded in 16 chunk DMAs at phase B
    start (keeps phase A under the SBUF cap).
"""

import numpy as np
import ml_dtypes
from contextlib import ExitStack

import concourse.bass as bass
import concourse.tile as tile
from concourse import bacc, mybir
from concourse.bass_utils import run_bass_kernel_spmd

F32 = mybir.dt.float32
BF16 = mybir.dt.bfloat16
Alu = mybir.AluOpType
Act = mybir.ActivationFunctionType

BSZ, SEQ, DM = 4, 2048, 1024
NH, DH = 16, 64
HPC = 8              # heads per core
HD = HPC * DH        # 512 = per-core slice of the head dim
NCORES = 8
SCALE = 1.0 / (DH ** 0.5)
LN_EPS = 1e-5

NT = SEQ // 128      # 16 seq tiles of 128
NDC = DM // 128      # 8 d_model chunks of 128
NHC = HD // 128      # 4 per-core head-dim chunks of 128
HVW = 128            # hv columns per head: 64 v-dims + ones col + pad to 128
                     # (128-wide stationary operands trigger FWL so the AV
                     # matmuls pipeline at ~220ns instead of ~378ns isolated)


def _mha_tile(ctx, tc, dq, dmask, dwq, dwk, dwv, dwo, dident, dgamma, dbeta,
              dqn, dpart, parts="full"):
    nc = tc.nc
    do_qkv = parts in ("qkv", "attn", "full")
    do_attn = parts in ("attn", "full")
    do_o = parts == "full"

    persist = ctx.enter_context(tc.tile_pool(name="persist", bufs=1))
    ident_sb = persist.tile([128, 128], BF16)
    nc.sync.dma_start(out=ident_sb, in_=dident)
    eps_sb = persist.tile([128, 1], F32)
    nc.vector.memset(eps_sb, LN_EPS)

    hqT = persist.tile([128, NHC * SEQ], BF16)      # [hd-chunk part, seq]
    hkT = persist.tile([128, NHC * SEQ], BF16)
    hv = persist.tile([128, NT * HPC * HVW], BF16)  # [j part, per-jc 8*65]
    vecT = persist.tile([128, NHC * SEQ], BF16)     # [hd-chunk part, i]
    wo_sb = persist.tile([128, NHC * DM], BF16)

    gamma_sb = beta_sb = None
    if dgamma is not None:
        gamma_sb = persist.tile([128, DM], F32)
        beta_sb = persist.tile([128, DM], F32)
        nc.sync.dma_start(out=gamma_sb, in_=bass.AP(
            tensor=dgamma.tensor, offset=dgamma.offset,
            ap=[[0, 128]] + list(dgamma.ap)))
        nc.sync.dma_start(out=beta_sb, in_=bass.AP(
            tensor=dbeta.tensor, offset=dbeta.offset,
            ap=[[0, 128]] + list(dbeta.ap)))

    # ---------------- Phase A: LN + transpose + QKV projections -------------
    with tc.tile_pool(name="phA", bufs=1) as pA, \
         tc.tile_pool(name="qtiles", bufs=2) as qpool, \
         tc.tile_pool(name="qnbf", bufs=2) as qnbfpool, \
         tc.tile_pool(name="stats", bufs=6) as spool, \
         tc.tile_pool(name="psT", bufs=2, space="PSUM") as psT, \
         tc.tile_pool(name="psQK", bufs=3, space="PSUM") as psQK:

        qnT = pA.tile([128, NDC * SEQ], BF16)       # [dm-chunk part, seq]
        wq_sb = pA.tile([128, NDC * HD], BF16)
        wk_sb = pA.tile([128, NDC * HD], BF16)
        wv_sb = pA.tile([128, NDC * HD], BF16)

        for tb in range(NT // 4):
            # one merged DMA loads 4 seq tiles
            rows4 = slice(tb * 512, (tb + 1) * 512)
            qt = qpool.tile([128, 4, DM], F32, tag="qt")
            nc.sync.dma_start(
                out=qt, in_=dq[rows4, :].rearrange("(a p) m -> p a m", p=128))
            if tb == 0:
                # weights are first needed by QKV(tb=0) ~30us in
                for w_sb, dw in ((wq_sb, dwq), (wk_sb, dwk), (wv_sb, dwv)):
                    nc.sync.dma_start(
                        out=w_sb.rearrange("p (dc hd) -> p dc hd", hd=HD),
                        in_=dw.rearrange("(dc p) hd -> p dc hd", p=128))
            qnbf = qnbfpool.tile([128, 4, DM], BF16, tag="qnbf")
            for k in range(4):
                st = spool.tile([128, 2, 6], F32, tag="st")
                nc.vector.bn_stats(out=st[:, 0, :], in_=qt[:, k, 0:512])
                nc.vector.bn_stats(out=st[:, 1, :], in_=qt[:, k, 512:1024])
                mv = spool.tile([128, 2], F32, tag="mv")
                nc.vector.bn_aggr(out=mv, in_=st)
                # rstd = 1/sqrt(var+eps): Sqrt on ACT (single table set for
                # all of phase A), reciprocal on DVE (~18-bit, plenty).
                std = spool.tile([128, 1], F32, tag="std")
                nc.scalar.activation(out=std, in_=mv[:, 1:2], func=Act.Sqrt,
                                     bias=eps_sb, scale=1.0)
                rstd = spool.tile([128, 1], F32, tag="rstd")
                nc.vector.reciprocal_approx_fast(out=rstd, in_=std)
                negmr = spool.tile([128, 1], F32, tag="negmr")
                nc.vector.tensor_tensor(out=negmr, in0=mv[:, 0:1], in1=rstd,
                                        op=Alu.mult)
                nc.vector.tensor_scalar_mul(negmr, negmr, -1.0)
                if gamma_sb is None:
                    nc.vector.tensor_scalar(out=qnbf[:, k, :], in0=qt[:, k, :],
                                            scalar1=rstd, scalar2=negmr,
                                            op0=Alu.mult, op1=Alu.add)
                else:
                    qnf = qt
                    nc.vector.tensor_scalar(out=qnf[:, k, :], in0=qt[:, k, :],
                                            scalar1=rstd, scalar2=negmr,
                                            op0=Alu.mult, op1=Alu.add)
                    nc.vector.tensor_tensor(out=qnf[:, k, :], in0=qnf[:, k, :],
                                            in1=gamma_sb, op=Alu.mult)
                    nc.vector.tensor_tensor(out=qnbf[:, k, :], in0=qnf[:, k, :],
                                            in1=beta_sb, op=Alu.add)
            nc.gpsimd.dma_start(
                out=dqn[rows4, :].rearrange("(a p) m -> p a m", p=128), in_=qnbf)
            for dc in range(NDC):
                pst = psT.tile([128, 512], BF16, tag="pst")
                for k in range(4):
                    nc.tensor.transpose(pst[:, k * 128:(k + 1) * 128],
                                        qnbf[:, k, dc * 128:(dc + 1) * 128],
                                        ident_sb)
                nc.scalar.copy(
                    out=qnT[:, dc * SEQ + tb * 512: dc * SEQ + (tb + 1) * 512],
                    in_=pst)

            # QKV for this seq block (sc == tb): overlaps the next block's LN
            if not do_qkv:
                continue
            sc = tb
            for w_sb, dstT in ((wq_sb, hqT), (wk_sb, hkT)):
                for hc in range(NHC):
                    ps = psQK.tile([128, 512], F32, tag="psqk")
                    for dc in range(NDC):
                        nc.tensor.matmul(
                            ps,
                            lhsT=w_sb[:, dc * HD + hc * 128: dc * HD + (hc + 1) * 128],
                            rhs=qnT[:, dc * SEQ + sc * 512: dc * SEQ + (sc + 1) * 512],
                            start=(dc == 0), stop=(dc == NDC - 1))
                    nc.scalar.copy(
                        out=dstT[:, hc * SEQ + sc * 512: hc * SEQ + (sc + 1) * 512],
                        in_=ps)
            for jc in range(4 * tb, 4 * tb + 4):
                ps = psQK.tile([128, HD], F32, tag="psv")
                for dc in range(NDC):
                    nc.tensor.matmul(
                        ps,
                        lhsT=qnT[:, dc * SEQ + jc * 128: dc * SEQ + jc * 128 + 128],
                        rhs=wv_sb[:, dc * HD:(dc + 1) * HD],
                        start=(dc == 0), stop=(dc == NDC - 1))
                blk = hv[:, jc * HPC * HVW:(jc + 1) * HPC * HVW]
                blk3 = blk.rearrange("p (h x) -> p h x", x=HVW)
                nc.scalar.copy(out=blk3[:, :, 0:DH],
                               in_=ps.rearrange("p (h x) -> p h x", x=DH))
        hv4 = hv.rearrange("p (j h x) -> p j h x", h=HPC, x=HVW)
        nc.vector.memset(hv4[:, :, :, DH:DH + 1], 1.0)
        nc.vector.memset(hv4[:, :, :, DH + 1:HVW], 0.0)

    # ---------------- Phase B: attention, ACT-bound ping-pong ---------------
    drecip = nc.dram_tensor(f"recip_scratch{nc.next_id()}", [HPC, SEQ], F32).ap()
    with tc.tile_pool(name="mk", bufs=1) as mkpool, \
         tc.tile_pool(name="pp", bufs=4) as ppool, \
         tc.tile_pool(name="stg", bufs=2) as stpool, \
         tc.tile_pool(name="den", bufs=1) as denpool, \
         tc.tile_pool(name="sps", bufs=1, space="PSUM") as spsum, \
         tc.tile_pool(name="vps", bufs=1, space="PSUM") as vpsum:
        # whole mask resident [j-part, jc, i]; 16 chunk DMAs stream in jc order
        mask_all = mkpool.tile([128, NT, SEQ], BF16)
        for c in range(NT):
            nc.sync.dma_start(out=mask_all[:, c, :],
                              in_=dmask[c * 128:(c + 1) * 128, :])
        nc.sync.dma_start(
            out=wo_sb.rearrange("p (hc m) -> p hc m", m=DM),
            in_=dwo.rearrange("(hc p) m -> p hc m", p=128))
        if not do_attn:
            nc.vector.memset(vecT, 0.0)

        def issue_av(pab, jc, vA, vB, ha, hb):
            va_l = hv[:, jc * HPC * HVW + ha * HVW: jc * HPC * HVW + ha * HVW + HVW]
            vb_l = hv[:, jc * HPC * HVW + hb * HVW: jc * HPC * HVW + hb * HVW + HVW]
            for n in range(2):
                osl = slice(n * 512, (n + 1) * 512)
                nc.tensor.matmul(vA[:, osl], lhsT=va_l, rhs=pab[:, n * 512:(n + 1) * 512],
                                 start=(jc == 0), stop=(jc == NT - 1))
                nc.tensor.matmul(vB[:, osl], lhsT=vb_l,
                                 rhs=pab[:, 1024 + n * 512: 1024 + (n + 1) * 512],
                                 start=(jc == 0), stop=(jc == NT - 1))

        for hp in range(HPC // 2 if do_attn else 0):
            ha, hb = 2 * hp, 2 * hp + 1
            for ih in range(2):
                # [128, 1024]: rows 0:64 vec, row 64 denominator, 65:128 pad
                # (M=128 so the stationary operand stays FWL-eligible)
                vA = vpsum.tile([128, 1024], F32, tag="vA")
                vB = vpsum.tile([128, 1024], F32, tag="vB")
                prev = None
                for jc in range(NT):
                    sA = spsum.tile([128, 1024], F32, tag="sA")
                    sB = spsum.tile([128, 1024], F32, tag="sB")
                    kslice = slice(hp * SEQ + jc * 128, hp * SEQ + (jc + 1) * 128)
                    for n in range(2):
                        qsl = slice(hp * SEQ + ih * 1024 + n * 512,
                                    hp * SEQ + ih * 1024 + (n + 1) * 512)
                        osl = slice(n * 512, (n + 1) * 512)
                        nc.tensor.matmul(sA[:, osl], lhsT=hkT[0:64, kslice],
                                         rhs=hqT[0:64, qsl], start=True, stop=True)
                        nc.tensor.matmul(sB[:, osl], lhsT=hkT[64:128, kslice],
                                         rhs=hqT[64:128, qsl], start=True, stop=True)
                    # PE pipeline: AV for jc-1 sits behind QK(jc); it becomes
                    # ready (mask done) while exp(jc) occupies ACT.
                    if prev is not None:
                        issue_av(prev[0], prev[1], vA, vB, ha, hb)
                    pab = ppool.tile([128, 2048], BF16, tag="pab")
                    nc.scalar.activation(out=pab[:, 0:1024], in_=sA,
                                         func=Act.Exp, scale=SCALE)
                    nc.scalar.activation(out=pab[:, 1024:2048], in_=sB,
                                         func=Act.Exp, scale=SCALE)
                    mk = mask_all[:, jc, ih * 1024:(ih + 1) * 1024]
                    mk_bc = bass.AP(tensor=mk.tensor, offset=mk.offset,
                                    ap=[list(mk.ap[0]), [0, 2]] + [list(a) for a in mk.ap[1:]])
                    nc.vector.tensor_tensor(out=pab, in0=pab, in1=mk_bc,
                                            op=Alu.mult)
                    prev = (pab, jc)
                issue_av(prev[0], prev[1], vA, vB, ha, hb)
                isl = slice(hp * SEQ + ih * 1024, hp * SEQ + (ih + 1) * 1024)
                dsl = slice(ih * 1024, (ih + 1) * 1024)
                # denominators (psum row 64) staged to SBUF (DMA cannot read
                # PSUM) then DMA'd to den_ih partitions 0/1; head-b vec rows
                # must move from psum partitions 0:64 to sbuf 64:128 -- both
                # are cross-partition moves only a DMA can do. Normalization
                # is per-ih so the final tail before the O-projection is one
                # [128, 1024] chunk, not a whole head-pair.
                den_ih = denpool.tile([2, 1024], F32, tag="den")
                dsa = stpool.tile([65, 1024], F32, tag="dsa")
                dsb = stpool.tile([65, 1024], F32, tag="dsb")
                nc.vector.tensor_copy(out=dsa[64:65, :], in_=vA[64:65, :])
                nc.vector.tensor_copy(out=dsb[64:65, :], in_=vB[64:65, :])
                nc.sync.dma_start(out=den_ih[0:1, :], in_=dsa[64:65, :])
                nc.sync.dma_start(out=den_ih[1:2, :], in_=dsb[64:65, :])
                nc.vector.tensor_copy(out=vecT[0:64, isl], in_=vA[0:64, :])
                stage = stpool.tile([64, 1024], BF16, tag="stg")
                nc.vector.tensor_copy(out=stage, in_=vB[0:64, :])
                nc.gpsimd.dma_start(out=vecT[64:128, isl], in_=stage)
                # normalize this chunk while the next chunk's attention runs
                recip_ih = denpool.tile([2, 1024], F32, tag="recip")
                nc.vector.reciprocal_approx_fast(out=recip_ih, in_=den_ih)
                nc.sync.dma_start(out=drecip[ha:hb + 1, dsl], in_=recip_ih)
                bc_ih = denpool.tile([128, 1024], F32, tag="bc")
                for h, lo in ((ha, 0), (hb, 64)):
                    row = drecip[h:h + 1, dsl]
                    nc.sync.dma_start(
                        out=bc_ih[lo:lo + 64, :],
                        in_=bass.AP(tensor=row.tensor, offset=row.offset,
                                    ap=[[0, 64]] + list(row.ap[1:])))
                nc.vector.tensor_tensor(out=vecT[:, isl], in0=vecT[:, isl],
                                        in1=bc_ih, op=Alu.mult)

    # ---------------- Phase C: O-projection ---------------------------------
    with tc.tile_pool(name="po", bufs=4, space="PSUM") as opool, \
         tc.tile_pool(name="outs", bufs=2) as outpool:
        for ob in range(NT // 4):
            outt = outpool.tile([128, 4, DM], F32, tag="outt")
            if not do_o:
                nc.vector.memset(outt, 0.0)
                nc.sync.dma_start(
                    out=dpart[ob * 512:(ob + 1) * 512, :]
                        .rearrange("(a p) m -> p a m", p=128),
                    in_=outt)
                continue
            for k in range(4):
                it = ob * 4 + k
                for mc in range(2):
                    po = opool.tile([128, 512], F32, tag="po")
                    for hc in range(NHC):
                        nc.tensor.matmul(
                            po,
                            lhsT=vecT[:, hc * SEQ + it * 128: hc * SEQ + (it + 1) * 128],
                            rhs=wo_sb[:, hc * DM + mc * 512: hc * DM + (mc + 1) * 512],
                            start=(hc == 0), stop=(hc == NHC - 1))
                    nc.vector.tensor_copy(out=outt[:, k, mc * 512:(mc + 1) * 512],
                                          in_=po)
            nc.sync.dma_start(
                out=dpart[ob * 512:(ob + 1) * 512, :]
                    .rearrange("(a p) m -> p a m", p=128),
                in_=outt)


_NC_CACHE = {}


def _build(gamma_trivial, repeat=1, parts="full"):
    key = (bool(gamma_trivial), repeat, parts)
    if key in _NC_CACHE:
        return _NC_CACHE[key]
    nc = bacc.Bacc("TRN2", target_bir_lowering=False, debug=False,
                   num_devices=NCORES)
    dq = nc.dram_tensor("q", [SEQ, DM], F32, kind="ExternalInput").ap()
    dmask = nc.dram_tensor("maskt", [SEQ, SEQ], BF16, kind="ExternalInput").ap()
    dwq = nc.dram_tensor("wq", [DM, HD], BF16, kind="ExternalInput").ap()
    dwk = nc.dram_tensor("wk", [DM, HD], BF16, kind="ExternalInput").ap()
    dwv = nc.dram_tensor("wv", [DM, HD], BF16, kind="ExternalInput").ap()
    dwo = nc.dram_tensor("wo", [HD, DM], BF16, kind="ExternalInput").ap()
    dident = nc.dram_tensor("ident", [128, 128], BF16, kind="ExternalInput").ap()
    dgamma = dbeta = None
    if not gamma_trivial:
        dgamma = nc.dram_tensor("gamma", [DM], F32, kind="ExternalInput").ap()
        dbeta = nc.dram_tensor("beta", [DM], F32, kind="ExternalInput").ap()
    dqn = nc.dram_tensor("qn_out", [SEQ, DM], BF16, kind="ExternalOutput").ap()
    dpart = nc.dram_tensor("part_out", [SEQ, DM], F32, kind="ExternalOutput").ap()
    with tile.TileContext(nc) as tc:
        for _rep in range(repeat):
            with ExitStack() as ctx:
                _mha_tile(ctx, tc, dq, dmask, dwq, dwk, dwv, dwo, dident,
                          dgamma, dbeta, dqn, dpart, parts=parts)
    nc.compile()
    _NC_CACHE[key] = nc
    return nc


def _run(nc, in_maps, **kwargs):
    return run_bass_kernel_spmd(nc, in_maps, list(range(NCORES)), **kwargs)


def make_in_maps(q, attn_mask, Wq, Wk, Wv, Wo, gamma, beta, gamma_trivial):
    bf = ml_dtypes.bfloat16
    q = np.ascontiguousarray(np.asarray(q, dtype=np.float32))
    maskt = np.ascontiguousarray(
        (~np.asarray(attn_mask, dtype=bool)).T.astype(bf))
    Wq = np.asarray(Wq, dtype=np.float32)
    Wk = np.asarray(Wk, dtype=np.float32)
    Wv = np.asarray(Wv, dtype=np.float32)
    Wo = np.asarray(Wo, dtype=np.float32)
    ident = np.eye(128, dtype=bf)
    in_maps = []
    for c in range(NCORES):
        b, g = c // 2, c % 2
        cols = slice(g * HD, (g + 1) * HD)
        m = {
            "q": q[b],
            "maskt": maskt,
            "wq": np.ascontiguousarray(Wq[:, cols].astype(bf)),
            "wk": np.ascontiguousarray(Wk[:, cols].astype(bf)),
            "wv": np.ascontiguousarray(Wv[:, cols].astype(bf)),
            "wo": np.ascontiguousarray(Wo[cols, :].astype(bf)),
            "ident": ident,
        }
        if not gamma_trivial:
            m["gamma"] = np.asarray(gamma, dtype=np.float32)
            m["beta"] = np.asarray(beta, dtype=np.float32)
        in_maps.append(m)
    return in_maps


def kernel(q, attn_mask, Wq, Wk, Wv, Wo, gamma, beta):
    gamma_np = np.asarray(gamma, dtype=np.float32)
    beta_np = np.asarray(beta, dtype=np.float32)
    gamma_trivial = bool(np.all(gamma_np == 1.0) and np.all(beta_np == 0.0))
    nc = _build(gamma_trivial)
    in_maps = make_in_maps(q, attn_mask, Wq, Wk, Wv, Wo, gamma_np, beta_np,
                           gamma_trivial)
    res = _run(nc, in_maps).results
    out = np.empty((BSZ, SEQ, DM), dtype=np.float32)
    for b in range(BSZ):
        out[b] = res[2 * b]["qn_out"].astype(np.float32)
        out[b] += res[2 * b]["part_out"]
        out[b] += res[2 * b + 1]["part_out"]
    return out


if __name__ == "__main__":
    rng = np.random.default_rng(0)
    ins = {
        "q": rng.standard_normal((BSZ, SEQ, DM), dtype=np.float32),
        "attn_mask": rng.integers(0, 2, (SEQ, SEQ)).astype(bool),
        "Wq": rng.standard_normal((DM, NH * DH), dtype=np.float32) * 0.03,
        "Wk": rng.standard_normal((DM, NH * DH), dtype=np.float32) * 0.03,
        "Wv": rng.standard_normal((DM, NH * DH), dtype=np.float32) * 0.03,
        "Wo": rng.standard_normal((NH * DH, DM), dtype=np.float32) * 0.03,
        "gamma": np.ones(DM, np.float32),
        "beta": np.zeros(DM, np.float32),
    }
    out = kernel(**ins)
    print("kernel ran, out shape", out.shape, out.dtype)
